# revision 11
# baseline (speedup 1.0000x reference)
"""MoE top-k routing + grouped down-proj GEMM + reduce-scatter for trn2 (8 cores).

Problem: intermediate_states [4, 2048, 1024] f16 (rank-sharded expanded-token
activations), w [4, 8, 1024, 2048] f16 (rank-sharded per-expert down-proj),
router_logits [1024, 8] f32, topk=2.  Output [4, 256, 2048] f16.

Strategy (expert-parallel): the sparse compute is, per expanded token tk with
selected expert e(tk): y_part[tk] = gate(tk) * (x_full[tk] @ w_full[e(tk)])
with x_full [TK, 4096] (rank dim folded into the contraction) and
w_full[e] [4096, 2048].  We give each of the 8 NeuronCores one expert:
core e runs a dense GEMM [cap, 4096] @ [4096, 2048] over the tokens routed to
expert e (host-gathered, zero-padded to `cap`), accumulating in fp32 PSUM over
32 K-subtiles, applying the fp32 routing gate as a per-partition scale at PSUM
eviction.  No collective is needed: the host gathers each token's topk partial
rows and adds them (tiny: 2*T rows).  Routing/top-k/softmax is done on host in
numpy (1024x8 logits - negligible).
"""

import numpy as np

R, T_TOK, TOPK, E = 4, 1024, 2, 8
I_PR, H = 1024, 2048
K = R * I_PR            # 4096 contraction
P = 128
KSUB = K // P           # 32
NF = 512                # matmul free-dim (one PSUM bank of fp32)
NH = H // NF            # 4
N_CORES = 8
CAP_DEFAULT = 384       # token capacity per expert (3 tiles of 128)
CAP_MAX_SINGLE = 384    # largest cap per launch (o-tile bufs == ntok == 3)

_prog_cache: dict[int, object] = {}


def _build_program(cap: int):
    import concourse.mybir as mybir
    import concourse.tile as tile
    from concourse import bacc

    f16 = mybir.dt.float16
    f32 = mybir.dt.float32
    ntok = cap // P

    nc = bacc.Bacc(
        "TRN2",
        target_bir_lowering=False,
        debug=False,
        num_devices=N_CORES,
    )
    xT = nc.declare_dram_parameter("xT", [KSUB, P, cap], f16, isOutput=False)
    wk = nc.declare_dram_parameter("wk", [KSUB, P, H], f16, isOutput=False)
    gs = nc.declare_dram_parameter("gs", [P, ntok], f32, isOutput=False)
    ho = nc.declare_dram_parameter("ho", [ntok, P, H], f16, isOutput=True)

    with tile.TileContext(nc) as tc:
        with tc.tile_pool(name="sb", bufs=1) as sb, \
             tc.tile_pool(name="ps", bufs=2, space="PSUM") as psp:
            xt, wt = [], []
            for k in range(KSUB):
                x_t = sb.tile([P, cap], f16, name=f"x{k}", tag=f"x{k}", bufs=1)
                nc.sync.dma_start(x_t[:], xT[k, :, :])
                w_t = sb.tile([P, H], f16, name=f"w{k}", tag=f"w{k}", bufs=1)
                nc.sync.dma_start(w_t[:], wk[k, :, :])
                xt.append(x_t)
                wt.append(w_t)
            # gates: one DMA into g_raw, then a scalar-engine copy into g2.
            # Evictions read g2, so their gate dependency is ACT-engine-local
            # and each eviction activation carries only the PE wait (walrus
            # allows a single sync-wait on Activation instructions).
            g_raw = sb.tile([P, ntok], f32, name="g_raw", tag="g_raw", bufs=1)
            nc.sync.dma_start(g_raw[:], gs[:, :])
            g2 = sb.tile([P, ntok], f32, name="g2", tag="g2", bufs=1)
            nc.scalar.copy(g2[:], g_raw[:])

            def tok_group(toks):
                # one 4-bank fp32 PSUM tile per token tile; <=2 token tiles
                # in flight keeps this within the 8 banks
                ps = {
                    t: psp.tile([P, H], f32, name=f"ps_t{t}", tag="ps", bufs=2)
                    for t in toks
                }
                for k in range(KSUB):
                    for t in toks:
                        lhs = xt[k][:, t * P:(t + 1) * P]   # stationary [K=128, M=128]
                        for h in range(NH):
                            nc.tensor.matmul(
                                ps[t][:, h * NF:(h + 1) * NF],
                                lhsT=lhs,
                                rhs=wt[k][:, h * NF:(h + 1) * NF],
                                start=(k == 0),
                                stop=(k == KSUB - 1),
                            )
                for t in toks:
                    o_t = sb.tile([P, H], f16, name=f"o{t}", tag="o", bufs=3)
                    # fp32 gate applied exactly: out = f16(psum_f32 * gate_f32)
                    nc.scalar.activation(
                        o_t[:],
                        ps[t][:],
                        mybir.ActivationFunctionType.Copy,
                        scale=g2[:, t:t + 1],
                    )
                    nc.sync.dma_start(ho[t, :, :], o_t[:])

            t = 0
            while t + 2 <= ntok:
                tok_group([t, t + 1])
                t += 2
            if t < ntok:
                tok_group([t])
    nc.finalize()
    return nc


def _get_program(cap: int):
    if cap not in _prog_cache:
        _prog_cache[cap] = _build_program(cap)
    return _prog_cache[cap]


def prepare(inputs):
    """Host routing + per-core input construction.

    Returns (nc, launches, combine) where launches is a list of per-launch
    in_maps (one dict per core) and combine(list_of_results) -> final output.
    """
    x = np.asarray(inputs["intermediate_states"])          # [R, TK, I_PR] f16
    w = np.asarray(inputs["w"])                            # [R, E, I_PR, H] f16
    logits = np.asarray(inputs["router_logits"]).astype(np.float32)  # [T, E]
    topk = int(np.asarray(inputs["topk"]))

    T, E_ = logits.shape
    TK = T * topk
    assert x.shape == (R, TK, I_PR) and w.shape == (R, E_, I_PR, H) and E_ == E

    # --- host routing (numpy replica of jax.lax.top_k + softmax) ---
    idx = np.argsort(-logits, axis=-1, kind="stable")[:, :topk]      # [T, topk]
    vals = np.take_along_axis(logits, idx, axis=-1)
    mx = vals.max(-1, keepdims=True)
    gate = np.exp(vals - mx)
    gate = gate / gate.sum(-1, keepdims=True)                        # [T, topk] f32

    flat_e = idx.reshape(-1)                                         # expert of tk
    counts = np.bincount(flat_e, minlength=E)
    starts = np.zeros(E + 1, np.int64)
    starts[1:] = np.cumsum(counts)
    order = np.argsort(flat_e, kind="stable")                        # sort tks by expert

    cap_needed = int(np.ceil(max(counts.max(), 1) / P)) * P
    cap_launch = min(max(cap_needed, CAP_DEFAULT), CAP_MAX_SINGLE)
    n_launch = -(-cap_needed // cap_launch)
    cap_total = n_launch * cap_launch

    # global slot (row in assembled h) for each expanded token
    pos = np.empty(TK, np.int64)
    pos[order] = (np.arange(TK) - starts[flat_e[order]]
                  + cap_total * flat_e[order].astype(np.int64))

    xf = np.ascontiguousarray(x.transpose(1, 0, 2)).reshape(TK, K)   # [TK, 4096] f16
    g_flat = gate.reshape(TK)

    nc = _get_program(cap_launch)
    ntok_l = cap_launch // P

    launches = []
    for j in range(n_launch):
        in_maps = []
        for e in range(E):
            toks = order[starts[e]:starts[e + 1]][j * cap_launch:(j + 1) * cap_launch]
            c = len(toks)
            xTe = np.zeros((K, cap_launch), np.float16)
            gse = np.zeros((cap_launch,), np.float32)
            if c:
                xTe[:, :c] = xf[toks].T
                gse[:c] = g_flat[toks]
            in_maps.append({
                "xT": np.ascontiguousarray(xTe.reshape(KSUB, P, cap_launch)),
                "wk": np.ascontiguousarray(w[:, e].reshape(K, H)).reshape(KSUB, P, H),
                # [P, ntok]: column t holds the gates of token tile t
                "gs": np.ascontiguousarray(gse.reshape(ntok_l, P).T),
            })
        launches.append(in_maps)

    def combine(all_results):
        h_all = np.empty((E * cap_total, H), np.float16)
        for j, res in enumerate(all_results):
            for e in range(E):
                h_all[e * cap_total + j * cap_launch:
                      e * cap_total + (j + 1) * cap_launch] = \
                    res[e]["ho"].reshape(cap_launch, H)
        y = h_all[pos[0::topk]].astype(np.float32)
        for kk in range(1, topk):
            y += h_all[pos[kk::topk]].astype(np.float32)
        return y.astype(np.float16).reshape(R, T // R, H)

    return nc, launches, combine


def kernel(**inputs) -> np.ndarray:
    nc, launches, combine = prepare(inputs)
    from concourse.bass_utils import run_bass_kernel_spmd

    all_results = []
    for in_maps in launches:
        res = run_bass_kernel_spmd(nc, in_maps, core_ids=list(range(N_CORES)))
        all_results.append(res.results)
    return combine(all_results)
